# revision 38
# baseline (speedup 1.0000x reference)
"""Two-layer GCN (GCNConv x2 + ReLU) on 8 Trainium2 NeuronCores.

Strategy (aggregate-first, dinv-folded, fp8 layer-2 tables):
  - Nodes sharded by destination across 8 cores. Layer 1 aggregates RAW
    input features: each core gathers X'[src] rows per edge (X' = dinv*X,
    host-prescaled, bf16, laid out in SLICES slice tables), scatter-adds
    them into per-dst-block PSUM accumulators (feature-major) via pure 0/1
    one-hot matmuls, then applies W1 + bias + ReLU + W2 per 128-node block
    on-chip. No X@W1 table phase at all.
  - dinv normalization folded exactly: one-hots are pure 0/1; source dinv
    lives in the tables; dst dinv is applied as a per-partition activation
    scale (relu is positively homogeneous): H3' table gets dinv^2, final
    output gets dinv. Biases enter as K=1 matmuls of b (x) sqrt(deg).
  - H3' = dinv^2*relu(AGG@W1 + b1*sqrt(deg))@W2 stored fp8 e4m3 at 64
    features -> the AllGather moves 3.2MB total instead of 12.8MB. After the
    AllGather each slice is expanded to 256B-row stride (dma_gather needs
    elem/stride multiples of 256B); layer 2 gathers those rows node-major.
  - Layer-2 one-hots are built as packed uint16 pairs on DVE (2-byte dtype
    keeps the 4x DVE mode, 77ns) and bitcast to fp8 [128,128]; layer-1
    one-hots are plain bf16 is_equal builds (94ns).
  - Self-loops are excluded from the edge lists (that removes exactly one
    full chunk per (block, slice-of-own-rows)): their contribution enters
    via identity matmuls against directly-read own-shard X' rows (layer 1)
    and the layer-1 H3 evac tiles still in SBUF (layer 2).
  - Edge chunks are packed CONTINUOUSLY per (supergroup of 7 dst blocks) x
    (source slice) run -- cells may straddle 128-edge chunk boundaries, each
    (cell, chunk) intersection getting its own one-hot column -- and a host
    node-permutation balances per-(block, slice) in-edge counts across the 8
    cores, so the shared SPMD schedule carries only ~3% padding. One SWDGE
    dma_gather covers up to 1024 edges (the hardware descriptor-ring cap).
    Chunks are consumed block-major so only ~2 blocks' PSUM accumulators are
    live; AllGathers/expansions are emitted with one-supergroup lag so the
    in-order Pool/SP queues never stall on their dependencies.
"""
import sys
sys.path.insert(0, '/opt/trn_rl_repo')
import numpy as np
import concourse.bass as bass
import concourse.bacc as bacc
import concourse.mybir as mybir
import bass_rust
from concourse.tile import TileContext
from concourse.tile_rust import add_dep_helper
from concourse.bass_utils import run_bass_kernel_spmd

dt = mybir.dt

NCORES = 8
SLICES = 3
WCHUNK = 8            # chunks per gather window (1024 idx = HW SWDGE ring cap)
SCRATCH = 16384        # SWDGE ring: 1024 descriptors (HW cap)
TAB8 = dt.float8e4     # table dtype (e4m3)
G1EXTRA = 0            # extra l1 gather buffers beyond windows-per-group
G2EXTRA = 2            # extra l2 gather buffers beyond windows-per-run
OHBUFS = 32            # one-hot tile ring
H2BUFS = 1             # W1-out PSUM ring
H3BUFS = 1             # W2-out PSUM ring
PKBUFS = 2             # park ring
EVBUFS = 8             # evac tile ring
ACCBUFS = 2            # PSUM buffers per L1 accumulator tag
PAD_DST = 999.0        # one-hot miss value for padding edges


def _np_dt(d):
    return mybir.dt.np(d)


# ---------------------------------------------------------------------------
# walrus in this toolchain rejects >1 attached sem wait on several opcodes;
# hoist extras into standalone InstEventSemaphore instructions just before.
def hoist_excess_waits(nc, max_attached=1):
    n_new = 0
    for f in nc.m.functions:
        for bb in f.blocks:
            insts = bb.instructions  # live list
            i = 0
            while i < len(insts):
                inst = insts[i]
                si = inst.sync_info
                if si is not None and inst.engine is not None:
                    waits = list(si.on_wait)
                    imm = [w for w in waits if w.wait_reg is None]
                    other = [w for w in waits if w.wait_reg is not None]
                    budget = max_attached - len(other)
                    if len(imm) > budget:
                        if budget > 0:
                            extra, keep = imm[:-budget], imm[-budget:]
                        else:
                            extra, keep = imm, []
                        for w in extra:
                            ev = mybir.InstEventSemaphore(
                                name=f"I-hoistw{n_new}", ins=[], outs=[])
                            ev.engine = inst.engine
                            h = bass_rust.SemaphoreHandle(name=w.ant_name, num=w.id)
                            bass_rust.wait_op(ev, h, w.wait_value, "sem-ge", True)
                            insts.insert(i, ev)
                            i += 1
                            n_new += 1
                        si.on_wait = other + keep
                i += 1
    return n_new


# ---------------------------------------------------------------------------
# node rebalance: permute nodes within each shard so that per-(block, slice)
# in-edge counts are flat across blocks (the shared SPMD chunk schedule is
# sized by the max over cores; flat per-core counts minimize that max).
def _rebalance(src, dst, N, NSH, TS, BARR, ncores):
    caps = np.full(TS, 128, np.int64)
    caps[TS - 1] = NSH - (TS - 1) * 128
    l_nat = np.arange(N, dtype=np.int64) % NSH
    jmem = np.searchsorted(BARR, l_nat, side='right') - 1  # slice membership
    d = np.stack([np.bincount(dst[jmem[src] == j], minlength=N)
                  for j in range(SLICES)]).astype(np.float64)   # [S, N]
    pos = np.empty(N, np.int64)
    for c in range(ncores):
        for jm in range(SLICES):
            blocks = np.arange(BARR[jm] // 128, BARR[jm + 1] // 128)
            ids = np.nonzero((np.arange(N) // NSH == c) & (jmem == jm))[0]
            dd = d[:, ids]                                # [S, n]
            nb = len(blocks)
            t = np.maximum(dd.sum(axis=1) / nb, 1.0)      # [S]
            order = np.argsort(-dd.sum(axis=0), kind='stable')
            ssum = np.zeros((SLICES, nb))
            cnt = np.zeros(nb, np.int64)
            bcaps = caps[blocks]
            for i in order:
                nrm = (ssum + dd[:, i:i + 1]) / t[:, None]
                cost = nrm.max(axis=0) + 0.05 * (nrm * nrm).sum(axis=0)
                cost[cnt >= bcaps] = np.inf
                bsel = int(np.argmin(cost))
                ssum[:, bsel] += dd[:, i]
                pos[ids[i]] = c * NSH + (blocks[bsel] * 128 + cnt[bsel])
                cnt[bsel] += 1
    return pos


# ---------------------------------------------------------------------------
# host-side graph preprocessing
def _prepare(x, edge_index, ncores):
    x = np.asarray(x, dtype=np.float32)
    N, D = x.shape
    NSH = (N + ncores - 1) // ncores            # nodes per shard (6250)
    TS = (NSH + 127) // 128                     # dst blocks per shard (49)
    GS = max(s for s in range(1, 9) if TS % s == 0)   # blocks per supergroup
    NG = TS // GS                               # supergroups per shard
    NSHP = TS * 128                             # padded shard rows

    # slice bounds in shard rows (multiples of GS*128); near-even split
    gb, rem = NG // SLICES, NG % SLICES
    parts = [2, 3, 2] if (NG, SLICES) == (7, 3) else [gb + (1 if i >= SLICES - rem else 0) for i in range(SLICES)]
    BOUNDS = [0]
    for p in parts:
        BOUNDS.append(BOUNDS[-1] + p * GS * 128)
    RSLS = [BOUNDS[i + 1] - BOUNDS[i] for i in range(SLICES)]
    assert all(ncores * r < 32768 for r in RSLS), RSLS
    BARR = np.array(BOUNDS)
    RARR = np.array(RSLS)

    src = edge_index[0].astype(np.int64)        # self-loops handled separately
    dst = edge_index[1].astype(np.int64)
    E = len(src)

    # permute nodes for balance; everything below works in the new id space
    newpos = _rebalance(src, dst, N, NSH, TS, BARR, ncores)
    x = x[np.argsort(newpos)]
    src = newpos[src]
    dst = newpos[dst]

    deg = np.bincount(dst, minlength=N).astype(np.float32) + 1.0  # + self loop
    dinv = 1.0 / np.sqrt(deg)

    # source table row (slice-local): node s -> shard c, local l;
    # slice j of l; row = c*RSL_j + (l - B_j)
    src_c, src_l = src // NSH, src % NSH
    src_j = (np.searchsorted(BARR, src_l, side='right') - 1).astype(np.int64)
    src_row = src_c * RARR[src_j] + (src_l - BARR[src_j])

    dst_c, dst_l = dst // NSH, dst % NSH
    dst_b = dst_l // 128
    dst_p = dst_l % 128

    # cell = (core, block, slice); shared schedule sized by max count per cell
    cell = (dst_c * TS + dst_b) * SLICES + src_j
    counts = np.bincount(cell, minlength=ncores * TS * SLICES)
    counts3 = counts.reshape(ncores, TS, SLICES)
    mx_cnt = counts3.max(axis=0)                        # [TS, SLICES]

    # continuous packing: per run (G, j), cells' edge spans (length mx_cnt)
    # are laid back-to-back; chunks of 128 descs may straddle cells. Each
    # (cell, chunk) intersection is a SEGMENT with its own one-hot column.
    cell_off = np.zeros((TS, SLICES), np.int64)   # desc offset of cell
    runs = {}                                     # (G, j) -> (k0, k1) chunks
    seg_block = []                                # per segment: block
    segs_by_block = {b: [] for b in range(TS)}    # b -> [(j, kk, col)]
    first_seg_of_cell = np.zeros((TS, SLICES), np.int64)
    first_chunk_of_cell = np.zeros((TS, SLICES), np.int64)
    k = 0                                         # chunk counter
    for G in range(NG):
        for j in range(SLICES):
            k0 = k
            d = 0                                 # desc offset within run
            for b in range(G * GS, (G + 1) * GS):
                cnt = int(mx_cnt[b, j])
                cell_off[b, j] = k0 * 128 + d
                if cnt > 0:
                    first_seg_of_cell[b, j] = len(seg_block)
                    first_chunk_of_cell[b, j] = k0 + d // 128
                    for kk in range(k0 + d // 128,
                                    k0 + (d + cnt - 1) // 128 + 1):
                        col = len(seg_block)
                        seg_block.append(b)
                        segs_by_block[b].append((j, kk, col))
                d += cnt
            k = k0 + (d + 127) // 128
            runs[(G, j)] = (k0, k)
    NCHT = k
    NSEG = len(seg_block)

    # rank of each edge within its cell (stable order)
    order = np.argsort(cell, kind='stable')
    starts = np.zeros(ncores * TS * SLICES + 1, np.int64)
    starts[1:] = np.cumsum(counts)
    rank = np.empty(E, np.int64)
    rank[order] = np.arange(E) - starts[cell[order]]

    slot = cell_off[dst_b, src_j] + rank        # flat desc slot per edge
    edge_col = (first_seg_of_cell[dst_b, src_j]
                + slot // 128 - first_chunk_of_cell[dst_b, src_j])

    idx_np = np.zeros((ncores, 128, NCHT * 8), np.int16)
    dstl_np = np.full((ncores, 128, NSEG), PAD_DST, np.float32)
    dpair_np = np.full((ncores, 128, NSEG), PAD_DST, np.float32)
    dpar_np = np.zeros((ncores, 128, NSEG), np.float32)
    for c in range(ncores):
        m = dst_c == c
        fsrc = np.zeros(NCHT * 128, np.int64)
        fsrc[slot[m]] = src_row[m]
        i16 = fsrc.astype(np.int16).reshape(-1, 16).T      # [16, NCHT*8]
        idx_np[c] = np.tile(i16, (8, 1))
        p_m = slot[m] % 128
        col_m = edge_col[m]
        dstl_np[c][p_m, col_m] = dst_p[m]
        dpair_np[c][p_m, col_m] = np.floor(dst_p[m] / 2.0)
        dpar_np[c][p_m, col_m] = np.where(dst_p[m] % 2 == 0, 56.0, 14336.0)

    # X' tables per slice (bf16), row = c*RSL_j + (l - B_j); pad rows zero
    f8 = _np_dt(TAB8)
    bf16 = _np_dt(dt.bfloat16)
    xp = (x * dinv[:, None]).astype(np.float32)
    Xs = []
    for j in range(SLICES):
        t = np.zeros((ncores * RSLS[j], D), np.float32)
        for c in range(ncores):
            l0, l1 = BOUNDS[j], BOUNDS[j + 1]
            n0 = c * NSH + l0
            n1 = min(c * NSH + min(l1, NSH), N)
            if n1 > n0:
                t[c * RSLS[j]:c * RSLS[j] + (n1 - n0)] = xp[n0:n1]
        Xs.append(t.astype(bf16))

    # per-core own-shard X' rows (plain order) for the self-loop term
    Xown_np = np.zeros((ncores, NSHP, D), np.float32)
    for c in range(ncores):
        n0, n1 = c * NSH, min((c + 1) * NSH, N)
        Xown_np[c, :n1 - n0] = xp[n0:n1]
    Xown_np = Xown_np.astype(f8)

    # per-core dst-side scales
    dinv2_np = np.ones((ncores, 128, TS), np.float32)
    dinv1_np = np.ones((ncores, 128, TS), np.float32)
    recip_np = np.zeros((ncores, 1, NSHP), np.float32)
    for c in range(ncores):
        n0, n1 = c * NSH, min((c + 1) * NSH, N)
        dloc = np.ones(NSHP, np.float32)
        dloc[:n1 - n0] = dinv[n0:n1]
        dinv2_np[c] = (dloc ** 2).reshape(TS, 128).T
        dinv1_np[c] = dloc.reshape(TS, 128).T
        r = np.zeros(NSHP, np.float32)
        r[:n1 - n0] = 1.0 / dinv[n0:n1]
        recip_np[c, 0] = r

    iota64 = np.tile(np.arange(64, dtype=np.uint16)[None, :], (128, 1)).copy()
    iota = np.tile(np.arange(128, dtype=np.float32)[None, :], (128, 1)).copy()
    id128 = np.eye(128, dtype=np.float32)

    return dict(N=N, D=D, NSH=NSH, TS=TS, GS=GS, NG=NG, NSHP=NSHP,
                newpos=newpos,
                BOUNDS=BOUNDS, RSLS=RSLS, NCHT=NCHT, NSEG=NSEG, runs=runs,
                segs_by_block=segs_by_block,
                idx_np=idx_np, dstl_np=dstl_np, dpair_np=dpair_np,
                dpar_np=dpar_np, iota=iota,
                Xs=Xs, Xown_np=Xown_np, dinv2_np=dinv2_np,
                dinv1_np=dinv1_np, recip_np=recip_np,
                iota64=iota64, id128=id128)


# ---------------------------------------------------------------------------
def _build(cfg, F1, F2):
    D, TS, GS, NG = cfg['D'], cfg['TS'], cfg['GS'], cfg['NG']
    NSHP, NCHT, NSEG = cfg['NSHP'], cfg['NCHT'], cfg['NSEG']
    BOUNDS, RSLS = cfg['BOUNDS'], cfg['RSLS']
    runs = cfg['runs']
    segs_by_block = cfg['segs_by_block']
    KD = D // 128

    nc = bacc.Bacc(None, target_bir_lowering=False,
                   dynamic_dma_scratch_size=SCRATCH)
    Xs_d = [nc.declare_dram_parameter(f"Xs{j}", [NCORES * RSLS[j], D],
                                      dt.bfloat16, isOutput=False)
            for j in range(SLICES)]
    Xown_d = nc.declare_dram_parameter("Xown", [NSHP, D], TAB8, isOutput=False)
    W1_d = nc.declare_dram_parameter("W1", [D, F1], dt.bfloat16, isOutput=False)
    W2_d = nc.declare_dram_parameter("W2", [F1, F2], dt.bfloat16, isOutput=False)
    b1_d = nc.declare_dram_parameter("b1", [1, F1], dt.bfloat16, isOutput=False)
    b2_d = nc.declare_dram_parameter("b2", [1, F2], dt.bfloat16, isOutput=False)
    iota64_d = nc.declare_dram_parameter("iota64", [128, 64], dt.uint16, isOutput=False)
    iota_d = nc.declare_dram_parameter("iota", [128, 128], dt.bfloat16, isOutput=False)
    id128_d = nc.declare_dram_parameter("id128", [128, 128], TAB8, isOutput=False)
    idb_d = nc.declare_dram_parameter("idb", [128, 128], dt.bfloat16, isOutput=False)
    dstl_d = nc.declare_dram_parameter("dstl", [128, NSEG], dt.float32, isOutput=False)
    idx_d = nc.declare_dram_parameter("idx", [128, NCHT * 8], dt.int16, isOutput=False)
    dpair_d = nc.declare_dram_parameter("dpair", [128, NSEG], dt.float32, isOutput=False)
    dpar_d = nc.declare_dram_parameter("dpar", [128, NSEG], dt.float32, isOutput=False)
    dinv2_d = nc.declare_dram_parameter("dinv2", [128, TS], dt.float32, isOutput=False)
    dinv1_d = nc.declare_dram_parameter("dinv1", [128, TS], dt.float32, isOutput=False)
    recip_d = nc.declare_dram_parameter("recip", [1, NSHP], dt.bfloat16, isOutput=False)
    out_d = nc.declare_dram_parameter("out", [NSHP, F2], dt.bfloat16, isOutput=True)

    H3shs = [nc.dram_tensor(f"H3sh{j}", [RSLS[j], F2], TAB8)
             for j in range(SLICES)]
    H3tabs = [nc.dram_tensor(f"H3tab{j}", [NCORES * RSLS[j], F2], TAB8,
                             addr_space="Shared") for j in range(SLICES)]
    H3exp = [nc.dram_tensor(f"H3exp{j}", [NCORES * RSLS[j], 256], TAB8)
             for j in range(SLICES)]

    def blk_slice(b):
        for j in range(SLICES):
            if (b + 1) * 128 <= BOUNDS[j + 1]:
                return j
        raise AssertionError(b)

    max_run = max(k1 - k0 for (k1, k0) in ((b, a) for (a, b) in runs.values()))
    wpr = -(-max_run // WCHUNK)          # windows per run

    with TileContext(nc) as tc:
        with (
            tc.tile_pool(name="const", bufs=1) as cp,
            tc.tile_pool(name="l1gt", bufs=SLICES * wpr + G1EXTRA) as g1p,
            tc.tile_pool(name="l2gt", bufs=wpr + G2EXTRA) as g2p,
            tc.tile_pool(name="oh16", bufs=OHBUFS) as ohp,
            tc.tile_pool(name="evac", bufs=EVBUFS) as evp,
            tc.tile_pool(name="h3sb", bufs=1) as h3p,
            tc.tile_pool(name="park", bufs=PKBUFS) as pkp,
        ):
            # ---- constants / metadata resident in SBUF ----
            iota64_t = cp.tile([128, 64], dt.uint16, tag="iota64")
            nc.sync.dma_start(iota64_t[:], iota64_d[:])
            id_t = cp.tile([128, 128], TAB8, tag="id128")
            nc.sync.dma_start(id_t[:], id128_d[:])
            idb_t = cp.tile([128, 128], dt.bfloat16, tag="idb")
            nc.sync.dma_start(idb_t[:], idb_d[:])
            iota_t = cp.tile([128, 128], dt.bfloat16, tag="iota")
            nc.sync.dma_start(iota_t[:], iota_d[:])
            dstl_t = cp.tile([128, NSEG], dt.float32, tag="dstl")
            nc.sync.dma_start(dstl_t[:], dstl_d[:])
            W1_t = cp.tile([128, KD, F1], dt.bfloat16, tag="W1")
            nc.sync.dma_start(W1_t[:], W1_d[:].rearrange("(k p) f -> p k f", p=128))
            W2_t = cp.tile([F1, F2], dt.bfloat16, tag="W2")
            nc.sync.dma_start(W2_t[:], W2_d[:])
            b1_t = cp.tile([1, F1], dt.bfloat16, tag="b1")
            nc.sync.dma_start(b1_t[:], b1_d[:])
            b2_t = cp.tile([1, F2], dt.bfloat16, tag="b2")
            nc.sync.dma_start(b2_t[:], b2_d[:])
            xo_t = cp.tile([128, TS, KD, 128], TAB8, tag="Xown")
            nc.sync.dma_start(
                xo_t[:], Xown_d[:].rearrange("(t p) (k f) -> p t k f",
                                             p=128, k=KD))
            idx_t = cp.tile([128, NCHT * 8], dt.int16, tag="idx")
            c0 = min(runs[(0, SLICES - 1)][1] * 8, NCHT * 8)
            nc.sync.dma_start(idx_t[:, 0:c0], idx_d[:, 0:c0])
            nc.sync.dma_start(idx_t[:, c0:], idx_d[:, c0:])
            dpair_t = cp.tile([128, NSEG], dt.float32, tag="dpair")
            nc.sync.dma_start(dpair_t[:], dpair_d[:])
            dpar_t = cp.tile([128, NSEG], dt.float32, tag="dpar")
            nc.sync.dma_start(dpar_t[:], dpar_d[:])
            dinv2_t = cp.tile([128, TS], dt.float32, tag="dinv2")
            nc.sync.dma_start(dinv2_t[:], dinv2_d[:])
            dinv1_t = cp.tile([128, TS], dt.float32, tag="dinv1")
            nc.sync.dma_start(dinv1_t[:], dinv1_d[:])
            recip_t = cp.tile([1, NSHP], dt.bfloat16, tag="recip")
            nc.sync.dma_start(recip_t[:], recip_d[:])

            def make_oh1(kk):
                oh = ohp.tile([128, 128], dt.bfloat16, tag="ohb")
                nc.vector.tensor_scalar(
                    oh[:], iota_t[:], dstl_t[:, kk:kk + 1], None,
                    mybir.AluOpType.is_equal)
                return oh[:]

            def make_oh(kk):
                oh = ohp.tile([128, 64], dt.uint16, tag="oh16")
                nc.vector.tensor_scalar(
                    oh[:], iota64_t[:], dpair_t[:, kk:kk + 1],
                    dpar_t[:, kk:kk + 1],
                    mybir.AluOpType.is_equal, mybir.AluOpType.mult)
                return oh[:].bitcast(TAB8)

            def win_tile_slot(G, j, kk, tiles):
                """gather tile + slot for chunk kk of run (G, j)."""
                k0, _ = runs[(G, j)]
                w = (kk - k0) // WCHUNK
                return tiles[(G, j)][w], (kk - k0) % WCHUNK

            h3_writes = {j: [] for j in range(SLICES)}
            h3s_tiles = {}
            exps = {}

            # ================= layer 1 =================
            with (
                tc.tile_pool(name="l1ps", bufs=ACCBUFS, space="PSUM") as app1,
                tc.tile_pool(name="h2ps", bufs=H2BUFS, space="PSUM") as hpp,
                tc.tile_pool(name="h3ps", bufs=H3BUFS, space="PSUM") as tpp,
            ):
                gt_tiles = {}
                for G in range(NG):
                    # issue all gathers of this supergroup (both slices)
                    for j in range(SLICES):
                        k0, k1 = runs[(G, j)]
                        tiles = []
                        for o in range(k0, k1, WCHUNK):
                            m = min(WCHUNK, k1 - o)
                            gt = g1p.tile([128, WCHUNK, D], dt.bfloat16, tag="g1")
                            nc.gpsimd.dma_gather(
                                gt[:, 0:m, :], Xs_d[j][:],
                                idx_t[:, o * 8:(o + m) * 8],
                                num_idxs=m * 128, num_idxs_reg=m * 128,
                                elem_size=D)
                            tiles.append(gt)
                        gt_tiles[(G, j)] = tiles

                    # consume block-major: self term, then both slices' segs
                    for b in range(G * GS, (G + 1) * GS):
                        accA = app1.tile([128, 128], dt.float32,
                                         name=f"accA{b}", tag="accA")
                        accB = app1.tile([128, 128], dt.float32,
                                         name=f"accB{b}", tag="accB")
                        acc = [accA, accB]
                        segs = segs_by_block[b]
                        # self-loop: acc[k,n] += Xown[n,k] via identity rhs
                        for kc in range(KD):
                            nc.tensor.matmul(acc[kc][:], xo_t[:, b, kc, :],
                                             id_t[:], start=True,
                                             stop=(len(segs) == 0))
                        for i, (j, kk, col) in enumerate(segs):
                            gt, s = win_tile_slot(G, j, kk, gt_tiles)
                            ohap = make_oh1(col)
                            last = (i == len(segs) - 1)
                            for kc in range(KD):
                                nc.tensor.matmul(
                                    acc[kc][:],
                                    gt[:, s, kc * 128:(kc + 1) * 128],
                                    ohap, start=False, stop=last)

                        # evac cascade: AGG -> W1+b1 -> relu -> W2 -> *dinv^2
                        agg = evp.tile([128, KD, 128], dt.bfloat16, tag="agg")
                        nc.scalar.activation(agg[:, 0, :], accA[:],
                                             mybir.ActivationFunctionType.Copy)
                        nc.scalar.activation(agg[:, 1, :], accB[:],
                                             mybir.ActivationFunctionType.Copy)
                        h2 = hpp.tile([F1, 128], dt.float32, tag="h2")
                        for kc in range(KD):
                            nc.tensor.matmul(h2[:], W1_t[:, kc, :], agg[:, kc, :],
                                             start=(kc == 0), stop=False)
                        nc.tensor.matmul(h2[:], b1_t[:],
                                         recip_t[0:1, b * 128:(b + 1) * 128],
                                         start=False, stop=True)
                        h2s = evp.tile([F1, 128], dt.bfloat16, tag="h2s")
                        nc.scalar.activation(h2s[:], h2[:],
                                             mybir.ActivationFunctionType.Relu)
                        h3 = tpp.tile([128, F2], dt.float32, tag="h3")
                        nc.tensor.matmul(h3[:], h2s[:], W2_t[:],
                                         start=True, stop=True)
                        h3s = h3p.tile([128, F2], TAB8,
                                       name=f"h3s{b}", tag=f"h3s{b}")
                        nc.scalar.activation(h3s[:], h3[:],
                                             mybir.ActivationFunctionType.Copy,
                                             bias=0.0, scale=dinv2_t[:, b:b + 1])
                        h3s_tiles[b] = h3s
                        j_b = blk_slice(b)
                        r0 = b * 128 - BOUNDS[j_b]
                        w = nc.sync.dma_start(H3shs[j_b][r0:r0 + 128, :], h3s[:])
                        h3_writes[j_b].append(w)
                    del gt_tiles[(G, 0)], gt_tiles[(G, 1)]

                    # fire slice AllGather + expansion as soon as ready
                    for j in range(SLICES):
                        if (G + 1) * GS * 128 == BOUNDS[j + 1]:
                            cc = nc.gpsimd.collective_compute(
                                "AllGather", mybir.AluOpType.bypass,
                                replica_groups=[list(range(NCORES))],
                                ins=[H3shs[j][:]], outs=[H3tabs[j][:]])
                            for w in h3_writes[j]:
                                add_dep_helper(cc.ins, w.ins,
                                               reason="allgather reads H3 slice")
                            ex = nc.scalar.dma_start(H3exp[j][:, 0:F2], H3tabs[j][:])
                            add_dep_helper(ex.ins, cc.ins,
                                           reason="expand reads allgathered tab")
                            exps[j] = ex

            # ================= layer 2 =================
            with tc.tile_pool(name="l2ps", bufs=3, space="PSUM") as app2:
                parks = {}
                for j in range(SLICES):
                    for G in range(NG):
                        k0, k1 = runs[(G, j)]
                        tiles = []
                        for o in range(k0, k1, WCHUNK):
                            m = min(WCHUNK, k1 - o)
                            gt8 = g2p.tile([128, WCHUNK, 256], TAB8, tag="g2")
                            gi = nc.gpsimd.dma_gather(
                                gt8[:, 0:m, :], H3exp[j][:],
                                idx_t[:, o * 8:(o + m) * 8],
                                num_idxs=m * 128, num_idxs_reg=m * 128,
                                elem_size=256, elem_step=256)
                            add_dep_helper(gi.ins, exps[j].ins,
                                           reason="gather reads expanded tab")
                            tiles.append(gt8)
                        gtt = {(G, j): tiles}

                        for b in range(G * GS, (G + 1) * GS):
                            segs = [t for t in segs_by_block[b] if t[0] == j]
                            m_j = len(segs)
                            if j == 0:
                                a = app2.tile([128, F2], dt.float32,
                                              name=f"acc2_{b}_0", tag="acc2")
                                # self-loop: acc2[n,f] += h3s[n,f]
                                nc.tensor.matmul(a[:], id_t[:],
                                                 h3s_tiles[b][:],
                                                 start=True, stop=(m_j == 0))
                                for i, (_, kk, col) in enumerate(segs):
                                    gt8, s = win_tile_slot(G, j, kk, gtt)
                                    nc.tensor.matmul(
                                        a[:], make_oh(col), gt8[:, s, 0:F2],
                                        start=False, stop=(i == m_j - 1))
                                pk = pkp.tile([128, F2], dt.bfloat16,
                                              name=f"park{b}_0", tag=f"pk{b}")
                                nc.vector.tensor_copy(pk[:], a[:])
                                parks[b] = pk
                            elif j < SLICES - 1:
                                if m_j == 0:
                                    continue
                                a = app2.tile([128, F2], dt.float32,
                                              name=f"acc2_{b}_{j}", tag="acc2")
                                for i, (_, kk, col) in enumerate(segs):
                                    gt8, s = win_tile_slot(G, j, kk, gtt)
                                    nc.tensor.matmul(
                                        a[:], make_oh(col), gt8[:, s, 0:F2],
                                        start=(i == 0), stop=(i == m_j - 1))
                                pk = pkp.tile([128, F2], dt.bfloat16,
                                              name=f"park{b}_{j}", tag=f"pk{b}")
                                nc.vector.tensor_tensor(
                                    pk[:], a[:], parks[b][:],
                                    mybir.AluOpType.add)
                                parks[b] = pk
                            else:
                                a = app2.tile([128, F2], dt.float32,
                                              name=f"acc2_{b}_f", tag="acc2")
                                # inject parked partial via identity matmul
                                nc.tensor.matmul(a[:], idb_t[:], parks[b][:],
                                                 start=True, stop=False)
                                for i, (_, kk, col) in enumerate(segs):
                                    gt8, s = win_tile_slot(G, j, kk, gtt)
                                    nc.tensor.matmul(
                                        a[:], make_oh(col), gt8[:, s, 0:F2],
                                        start=False, stop=False)
                                # bias (b2 (x) sqrt(deg)) closes the group
                                nc.tensor.matmul(
                                    a[:], recip_t[0:1, b * 128:(b + 1) * 128],
                                    b2_t[:], start=False, stop=True)
                                ost = evp.tile([128, F2], dt.bfloat16, tag="ost")
                                nc.scalar.activation(
                                    ost[:], a[:],
                                    mybir.ActivationFunctionType.Copy,
                                    bias=0.0, scale=dinv1_t[:, b:b + 1])
                                nc.sync.dma_start(
                                    out_d[b * 128:(b + 1) * 128, :], ost[:])

    if not nc.is_finalized():
        nc.finalize()
    hoist_excess_waits(nc)
    return nc


# ---------------------------------------------------------------------------
def _kernel_impl(x, edge_index, W1, b1, W2, b2, ncores=NCORES):
    x = np.asarray(x, dtype=np.float32)
    edge_index = np.asarray(edge_index)
    W1 = np.asarray(W1, dtype=np.float32)
    b1 = np.asarray(b1, dtype=np.float32)
    W2 = np.asarray(W2, dtype=np.float32)
    b2 = np.asarray(b2, dtype=np.float32)
    N, D = x.shape
    F1 = W1.shape[1]
    F2 = W2.shape[1]

    cfg = _prepare(x, edge_index, ncores)
    nc = _build(cfg, F1, F2)

    bf16 = _np_dt(dt.bfloat16)
    in_maps = []
    for c in range(ncores):
        m = {f"Xs{j}": cfg['Xs'][j] for j in range(SLICES)}
        m.update({
            "Xown": cfg['Xown_np'][c],
            "W1": W1.astype(bf16),
            "W2": W2.astype(bf16),
            "b1": b1.reshape(1, F1).astype(bf16),
            "b2": b2.reshape(1, F2).astype(bf16),
            "iota64": cfg['iota64'],
            "iota": cfg['iota'].astype(bf16),
            "id128": cfg['id128'].astype(_np_dt(dt.float8e4)),
            "idb": cfg['id128'].astype(bf16),
            "dstl": cfg['dstl_np'][c],
            "idx": cfg['idx_np'][c],
            "dpair": cfg['dpair_np'][c],
            "dpar": cfg['dpar_np'][c],
            "dinv2": cfg['dinv2_np'][c],
            "dinv1": cfg['dinv1_np'][c],
            "recip": cfg['recip_np'][c].astype(bf16),
        })
        in_maps.append(m)
    res = run_bass_kernel_spmd(nc, in_maps, list(range(ncores)))

    NSH = cfg['NSH']
    outp = np.empty((N, F2), np.float32)
    for c in range(ncores):
        o = res.results[c]["out"]            # [NSHP, F2]
        n0 = c * NSH
        n1 = min(N, n0 + NSH)
        outp[n0:n1] = o[:n1 - n0].astype(np.float32)
    out = outp[cfg['newpos']]                # back to original node order
    return out, res, nc, cfg


def kernel(x, edge_index, W1, b1, W2, b2):
    out, _, _, _ = _kernel_impl(x, edge_index, W1, b1, W2, b2)
    return out
